# revision 12
# baseline (speedup 1.0000x reference)
"""Region-augmented embedding lookup (MeanEncoder) on 8 TRN2 NeuronCores.

Reference computation (per batch b, position l):
    out[b,l,0,:] = tanh( sum_{j=0..6} W[ seq_pad[b, l+j]*7 + j , :] ) * (seq[b,l]!=0)

Strategy: data parallel, W replicated, each core takes 2 of 16 sequences.

Device kernel, super-groups of 4 tiles (each tile = 122 output positions
from 128 gathered window positions):
  1. Four indirect DMAs gather per-token contiguous 7x128 blocks
     W[tok*7 : tok*7+7, :] into one [128, 4*896] SBUF tile (the TRN2
     indirect DMA consumes one index per dest partition and streams the
     partition row from that base address -- 3584B/descriptor).
  2. Shifted region-sum out[i] = sum_j G[i+j, seg_j] on the tensor
     engine: 7 fp32 matmuls, lhsT = identity slice ID[:, j:j+122]
     (stationary), rhs = the j-th 128-col segment of all 4 tiles
     (N=512 moving), PSUM-accumulated. Exact fp32.
  3. Per tile, one scalar-engine activation tanh(psum * mask) (exact:
     mask is 0/1), then ONE batched store DMA per group (488 rows).
Out-of-sequence window positions use token id 0 (= the reference pad).
"""

import numpy as np

import concourse.bass as bass
import concourse.tile as tile
from concourse import bacc, mybir
from concourse.bass_utils import run_bass_kernel_spmd

VOCAB = 50000
EMB = 128
RADIUS = 3
REGION = 7
B, L, C = 16, 2048, 1
NCORES = 8
SEQ_PER_CORE = B // NCORES           # 2
P = 128                              # gathered window positions per tile
TOUT = P - (REGION - 1)              # 122 output positions per tile
TILES_PER_SEQ = -(-L // TOUT)        # 17 (16*122=1952, last tile 96 rows)
NTILES = SEQ_PER_CORE * TILES_PER_SEQ  # 34
GRP = 4                              # tiles per super-group
BLK = REGION * EMB                   # 896


def _build_nc():
    nc = bacc.Bacc("TRN2", target_bir_lowering=False, debug=False)

    w = nc.declare_dram_parameter("w", [VOCAB * REGION, EMB], mybir.dt.float32, isOutput=False)
    gidx = nc.declare_dram_parameter("gidx", [P, NTILES], mybir.dt.int32, isOutput=False)
    mask = nc.declare_dram_parameter("mask", [P, NTILES], mybir.dt.float32, isOutput=False)
    ident = nc.declare_dram_parameter("ident", [P, P], mybir.dt.float32, isOutput=False)
    out = nc.declare_dram_parameter("out", [SEQ_PER_CORE * L, EMB], mybir.dt.float32, isOutput=True)

    # tile t -> (sequence s, within-seq k); groups of 4 consecutive tiles
    # within one sequence, the 17th tile of each sequence stands alone.
    groups = []
    for s in range(SEQ_PER_CORE):
        base = s * TILES_PER_SEQ
        groups += [[base + k0 + u for u in range(min(GRP, (TILES_PER_SEQ - 1) - k0))]
                   for k0 in range(0, TILES_PER_SEQ - 1, GRP)]
        groups.append([base + TILES_PER_SEQ - 1])  # ragged last tile (96 rows)

    from contextlib import ExitStack
    with tile.TileContext(nc) as tc, ExitStack() as ctx:
        const_pool = ctx.enter_context(tc.tile_pool(name="const", bufs=1))
        gpool = ctx.enter_context(tc.tile_pool(name="gather", bufs=3))
        ppool = ctx.enter_context(tc.tile_pool(name="psum", bufs=2, space="PSUM"))
        opool = ctx.enter_context(tc.tile_pool(name="out", bufs=3))

        gidx_sb = const_pool.tile([P, NTILES], mybir.dt.int32)
        mask_sb = const_pool.tile([P, NTILES], mybir.dt.float32)
        id_sb = const_pool.tile([P, P], mybir.dt.float32)
        nc.sync.dma_start(gidx_sb[:], gidx.ap())
        nc.sync.dma_start(mask_sb[:], mask.ap())
        nc.sync.dma_start(id_sb[:], ident.ap())

        for g_tiles in groups:
            ng = len(g_tiles)
            t0 = g_tiles[0]
            s, k0 = divmod(t0, TILES_PER_SEQ)
            row0 = s * L + k0 * TOUT
            nrows_last = min(TOUT, L - (k0 + ng - 1) * TOUT)
            tot_rows = (ng - 1) * TOUT + nrows_last

            gsb = gpool.tile([P, GRP * BLK], mybir.dt.float32, tag="g")
            for u, t in enumerate(g_tiles):
                nc.gpsimd.indirect_dma_start(
                    out=gsb[:, u * BLK:(u + 1) * BLK],
                    out_offset=None,
                    in_=w.ap(),
                    in_offset=bass.IndirectOffsetOnAxis(ap=gidx_sb[:, t: t + 1], axis=0),
                )
            psum = ppool.tile([TOUT, GRP * EMB], mybir.dt.float32, tag="ps")
            # rhs for offset j: the j-th 128-col segment of each tile's block
            gv = gsb[:].rearrange("p (u j e) -> p u j e", u=GRP, j=REGION)
            for j in range(REGION):
                nc.tensor.matmul(
                    out=psum[:, : ng * EMB],
                    lhsT=id_sb[:, j: j + TOUT],
                    rhs=gv[:, :ng, j, :],
                    start=(j == 0),
                    stop=(j == REGION - 1),
                )
            o = opool.tile([TOUT, GRP * EMB], mybir.dt.float32, tag="o")
            for u, t in enumerate(g_tiles):
                nc.scalar.activation(
                    o[:, u * EMB:(u + 1) * EMB], psum[:, u * EMB:(u + 1) * EMB],
                    mybir.ActivationFunctionType.Tanh,
                    scale=mask_sb[:TOUT, t: t + 1],
                )
            # one store: SBUF (i, u, e) -> DRAM rows row0 + u*TOUT + i
            if ng == 1:
                nc.sync.dma_start(out.ap()[row0: row0 + tot_rows, :], o[:tot_rows, :EMB])
            else:
                dst = out.ap()[row0: row0 + ng * TOUT, :].rearrange("(u i) e -> i u e", u=ng)
                src = o[:].rearrange("i (u e) -> i u e", u=GRP)[:, :ng, :]
                nc.sync.dma_start(dst, src)
    nc.compile()
    return nc


def _host_prep(seq, W):
    s = seq.reshape(B, L)
    mask_full = (s != 0).astype(np.float32)
    ident = np.eye(P, dtype=np.float32)

    in_maps = []
    for c in range(NCORES):
        gidx_r = np.zeros((P, NTILES), np.int32)
        mask_r = np.zeros((P, NTILES), np.float32)
        for t in range(NTILES):
            sq, k = divmod(t, TILES_PER_SEQ)
            b = c * SEQ_PER_CORE + sq
            q0 = k * TOUT
            v = q0 - RADIUS + np.arange(P)
            tok = np.where((v >= 0) & (v < L), s[b, np.clip(v, 0, L - 1)], 0)
            gidx_r[:, t] = tok.astype(np.int32) * REGION
            nrows = min(TOUT, L - q0)
            mask_r[:nrows, t] = mask_full[b, q0: q0 + nrows]
        in_maps.append({
            "w": np.ascontiguousarray(W),
            "gidx": gidx_r,
            "mask": mask_r,
            "ident": ident,
        })
    return in_maps


_NC_CACHE = None


def run(seq, W, trace=False, **spmd_kwargs):
    global _NC_CACHE
    if _NC_CACHE is None:
        _NC_CACHE = _build_nc()
    nc = _NC_CACHE
    in_maps = _host_prep(seq, W)
    res = run_bass_kernel_spmd(
        nc, in_maps, core_ids=list(range(NCORES)), trace=trace, **spmd_kwargs
    )
    outs = [r["out"] for r in res.results]                 # each [2*L, EMB]
    full = np.stack(outs, axis=0).reshape(B, L, EMB)[:, :, None, :]
    return full.astype(np.float32), res


def kernel(seq, W):
    out, _ = run(np.asarray(seq), np.asarray(W))
    return out
